# revision 15
# baseline (speedup 1.0000x reference)
"""Trainium2 Bass kernel for nn_BakeAugment.

Key insight: at QUALITY=15 the JPEG quantization table is q = Y_TABLE*(10/3)+1e-5
(min entry ~33.3), while the 8x8 DCT coefficients of any image with values in
[0,1] are bounded by |c| <= 8 (worst case |c|/q = 0.15 < 0.5).  Hence
round(dctp/q) == 0 for EVERY coefficient, rec == 0, and apply_jpeg() is the
CONSTANT image clip(ycbcr_to_rgb(0,0,0), 0, 1) = (0.0, 0.52914, 0.0).

Therefore (with s = shift*0.05):
  inp[:,c]  = min(clip(clip01(K_c + 0.03*gauss[:,c]) + s_c, 1e-8, inf)^0.9, 1)
              with K = (0, 0.52914, 0); the upper clip at 1 before pow is
              redundant with the final min(.,1) since pow is monotonic.
  target[:,c] = min(clip(x[:,c] + s_c, 1e-8, inf)^0.9, 1)
  dither is entirely unused.

Device program per core (2 images): 12 tiles of [128, 2048] f32
(6 from gauss -> inp, 6 from x -> target):
  load -> tensor_scalar fused ops -> Ln -> Exp(scale=0.9) -> min(1) -> store.
Pure data parallel across 8 NeuronCores, batch-sharded.
"""

import numpy as np

B, C, H, W = 16, 3, 512, 512
NCORES = 8
PER = B // NCORES  # 2 images per core
F = np.float32

SIGMA = 0.03
EPS = 1e-8
GAMMA = 0.9

# apply_jpeg constant output per channel, fp32-faithful to the reference:
# rec_yuv == 0 exactly; ycbcr_to_rgb gives (1.402*-0.5, 0.34414*0.5+0.71414*0.5,
# 1.772*-0.5) = (-0.701, 0.52914, -0.886); clip01 -> (0, 0.52914, 0).
K_G = float(F(0.0) - F(0.34414) * (F(0.0) - F(0.5)) - F(0.71414) * (F(0.0) - F(0.5)))
K_CH = (0.0, K_G, 0.0)


def _build_nc(s):
    """Build the per-core Bass program. s: per-channel shift*0.05 (3 floats,
    baked in as immediates at trace time)."""
    import concourse.bass as bass
    import concourse.mybir as mybir
    from concourse.tile import TileContext

    f32 = mybir.dt.float32
    Alu = mybir.AluOpType
    Act = mybir.ActivationFunctionType

    nc = bass.Bass(trn_type="TRN2", target_bir_lowering=False)

    xt = nc.dram_tensor("xt", [PER, C, H, W], f32, kind="ExternalInput")
    gg = nc.dram_tensor("gg", [PER, C, H, W], f32, kind="ExternalInput")
    to = nc.dram_tensor("to", [PER, C, H, W], f32, kind="ExternalOutput")
    io = nc.dram_tensor("io", [PER, C, H, W], f32, kind="ExternalOutput")

    NB = H // 128  # 4 row-bands per image
    FD = C * NB * W  # 6144 free elems per tile; channel c at [c*NB*W,(c+1)*NB*W)
    CW = NB * W

    # gen3 instructions accept a SINGLE sync-wait condition and the tile
    # scheduler does not split overflowing waits, so the program is shaped so
    # no instruction ever needs two:
    #  - every tile is allocated exactly once (all-fresh; 4 x 24KB loads +
    #    4 x 24KB workspaces = 192KB/partition out of 224KB);
    #  - one whole-image-x-stream DMA per direction: 4 HWDGE loads and
    #    4 SWDGE stores, so no DMA sem-lane is ever reused;
    #  - stores read only DVE/ACT-produced tiles (single DVE wait);
    #  - a warmup ACT op absorbs the const-bias preamble DMA dependency.
    with TileContext(nc) as tc:
        with tc.tile_pool(name="p", bufs=2 * PER) as pool:
            warm = pool.tile([128, 1], f32, tag="warm")
            nc.scalar.activation(
                warm[:], nc.const_aps.tensor(0.0, (128, 1)), Act.Exp
            )

            def emit(src_dram, dst_dram, is_inp):
                a = pool.tile([128, FD], f32, tag="a")
                nc.sync.dma_start(
                    out=a[:].rearrange("p (c rw) -> p c rw", c=C),
                    in_=src_dram.rearrange("c (p r) w -> p c (r w)", p=128),
                )
                w_ = pool.tile([128, FD], f32, tag="w")
                for c in range(C):
                    ws = w_[:, c * CW:(c + 1) * CW]
                    as_ = a[:, c * CW:(c + 1) * CW]
                    if is_inp:
                        # t = (gauss * 0.03) + K_c  (K_r = K_b = 0: exact no-op)
                        nc.vector.tensor_scalar(
                            out=ws, in0=as_, scalar1=SIGMA, scalar2=K_CH[c],
                            op0=Alu.mult, op1=Alu.add,
                        )
                        # clip01 of the jpeg-const + noise image
                        nc.vector.tensor_scalar(
                            out=ws, in0=ws, scalar1=0.0, scalar2=1.0,
                            op0=Alu.max, op1=Alu.min,
                        )
                        # u = max(t + s_c, eps); upper clip pre-pow via min1
                        nc.vector.tensor_scalar(
                            out=ws, in0=ws, scalar1=float(s[c]), scalar2=EPS,
                            op0=Alu.add, op1=Alu.max,
                        )
                    else:
                        # u = max(x + s_c, eps)
                        nc.vector.tensor_scalar(
                            out=ws, in0=as_, scalar1=float(s[c]), scalar2=EPS,
                            op0=Alu.add, op1=Alu.max,
                        )
                    nc.scalar.activation(ws, ws, Act.Ln)
                    nc.scalar.activation(ws, ws, Act.Exp, scale=GAMMA)
                    # w = min(u^0.9, 1) == reference's clip(.,1) pre- or
                    # post-pow (pow is monotonic, 1^0.9 = 1).  Last op on DVE
                    # so the store needs only a single DVE wait.
                    nc.vector.tensor_scalar_min(out=ws, in0=ws, scalar1=1.0)
                nc.gpsimd.dma_start(
                    out=dst_dram.rearrange("c (p r) w -> p c (r w)", p=128),
                    in_=w_[:].rearrange("p (c rw) -> p c rw", c=C),
                )

            for b in range(PER):
                emit(gg[b], io[b], True)
                emit(xt[b], to[b], False)

    _legalize_tail_drain(nc)
    return nc


def _legalize_tail_drain(nc):
    """gen3 allows one sync-wait per instruction (EventSemaphore: two), but
    the TileContext tail drain carries one wait per outstanding proc.  In
    this program the drain's ACT and DMAHW (load-lane) waits are transitively
    implied by its DVE wait (the last DVE min1 directly waits the last ACT
    op, and every load lane was waited on by a DVE TS op), so: keep DVE on
    the drain, move each DMASW (store-lane) wait onto a distinct round-1
    barrier EventSemaphore that follows the drain (each has a free slot and
    executes before the semaphores are cleared), and drop the rest."""
    for blk in nc.m.functions[0].blocks:
        insts = blk.instructions
        for idx, inst in enumerate(insts):
            si = inst.sync_info
            if si is None or not si.on_wait or len(si.on_wait) <= 1:
                continue
            if type(inst).__name__ == "InstEventSemaphore":
                continue  # 2-wait capacity; hosts patched below
            waits = list(si.on_wait)
            names = [w.ant_name for w in waits]
            assert str(inst.opcode).endswith("Drain"), (inst.name, names)
            keep = [w for w in waits if w.ant_name.startswith("DVE")]
            move = [w for w in waits if w.ant_name.startswith("DMASW")]
            dropped = [w.ant_name for w in waits
                       if not (w.ant_name.startswith(("DVE", "DMASW")))]
            assert keep and all(n.startswith(("Activation", "DMAHW"))
                                for n in dropped), (names,)
            inst.sync_info.on_wait = keep
            hosts = []
            for later in insts[idx + 1:]:
                lsi = later.sync_info
                if (type(later).__name__ == "InstEventSemaphore"
                        and lsi is not None and len(lsi.on_wait or []) == 1
                        and later.name.startswith("barrier_")):
                    hosts.append(later)
                if len(hosts) >= len(move):
                    break
            assert len(hosts) >= len(move), (len(hosts), len(move))
            for host, w in zip(hosts, move):
                host.sync_info.on_wait = list(host.sync_info.on_wait) + [w]
    return nc


_bench = [None]


def kernel(x, dither, gauss, shift):
    from concourse.bass_utils import run_bass_kernel_spmd

    x = np.ascontiguousarray(np.asarray(x, dtype=np.float32))
    gauss = np.ascontiguousarray(np.asarray(gauss, dtype=np.float32))
    shift = np.asarray(shift, dtype=np.float32).reshape(C)
    s = (shift * F(0.05)).astype(np.float32)

    nc = _build_nc(s)

    in_maps = []
    for i in range(NCORES):
        sl = slice(i * PER, (i + 1) * PER)
        in_maps.append({"xt": x[sl], "gg": gauss[sl]})

    res = run_bass_kernel_spmd(nc, in_maps, core_ids=list(range(NCORES)))
    _bench[0] = res  # stash for test harness introspection

    inp = np.empty((B, C, H, W), dtype=np.float32)
    tgt = np.empty((B, C, H, W), dtype=np.float32)
    for i in range(NCORES):
        sl = slice(i * PER, (i + 1) * PER)
        inp[sl] = res.results[i]["io"]
        tgt[sl] = res.results[i]["to"]
    return inp, tgt
